# revision 15
# baseline (speedup 1.0000x reference)
"""ClusterMemory (mode='CM') forward + memory-bank momentum update, Trainium2.

Strategy (8 NeuronCores, SPMD):
  - features [400000, 256] sharded row-wise: 50000 rows/core, shipped to the
    device pre-transposed ([256, n]) and cast to bf16 on the host so the
    device streams contiguous tiles straight into the matmul.
  - Per core: logits_tile = x_norm @ ft_shard (bf16 matmul, fp32 PSUM),
    fused exp(logit/TEMP) + row-sum on ScalarE (accum_out). Inputs are
    L2-normalized so |logit| <= 1/TEMP = 20 and sumexp fits fp32 with no
    max-subtraction pass.
  - Device returns only [2, 128, NT] partial row-sums per core (25.6 KB);
    host combines logsumexp in fp64, computes the target logits and the
    sequential 256-row momentum scatter (tiny) exactly in fp32, and
    assembles the full new_features.
"""

import numpy as np
import ml_dtypes

TEMP = 0.05
MOMENTUM = 0.2
B, D, N = 256, 256, 400000
NCORES = 8
NSHARD = N // NCORES        # 50000 feature rows per core
TILE_COLS = 2048            # columns per PSUM tile / ACT op
NT = 25                     # tiles per core
NPAD = TILE_COLS * NT       # 51200 (zero-padded; pad cols contribute exp(0)=1)
BF16 = ml_dtypes.bfloat16

_NC_CACHE = None
LAST_EXEC_NS = None


def _build_nc(tile_cols=TILE_COLS, nt=NT, bass_kwargs=None):
    import concourse.bass as bass
    import concourse.mybir as mybir
    import concourse.tile as tile
    from concourse import bacc
    from concourse._compat import get_trn_type

    npad = tile_cols * nt
    # Bacc (not plain Bass): its compile() runs generate_event_semaphores,
    # which splits multi-wait DMAs to satisfy TRN2's 1-wait-per-instruction
    # limit — walrus rejects the raw form with "Too many sync wait commands".
    nc = bacc.Bacc(
        get_trn_type() or "TRN2",
        target_bir_lowering=False,
        debug=False,
        num_devices=NCORES,
        **(bass_kwargs or {}),
    )
    bf16 = mybir.dt.bfloat16
    f32 = mybir.dt.float32

    xt = nc.declare_dram_parameter("xt", [2, 128, B], bf16, isOutput=False)
    # host layout: ft[p, kc, n] = features_shard.T[kc*128 + p, n]
    ft = nc.declare_dram_parameter("ft", [128, 2, npad], bf16, isOutput=False)
    out = nc.declare_dram_parameter("partials", [2, 128, nt], f32, isOutput=True)

    with tile.TileContext(nc) as tc:
        with (
            tc.tile_pool(name="const", bufs=1) as const_pool,
            tc.tile_pool(name="ftp", bufs=4) as ft_pool,
            tc.tile_pool(name="stat", bufs=1) as stat_pool,
            tc.tile_pool(name="psum", bufs=2, space=bass.MemorySpace.PSUM) as psum_pool,
        ):
            # x.T resident in SBUF: [d-chunk partitions, kc, b]
            xt_sb = const_pool.tile([128, 2, B], bf16)
            nc.sync.dma_start(xt_sb[:, 0, :], xt[0])
            nc.sync.dma_start(xt_sb[:, 1, :], xt[1])

            # warm the ACT exp table before the pipeline needs it
            warm = const_pool.tile([128, 1], f32)
            nc.gpsimd.memset(warm[:], 0.0)
            nc.scalar.activation(
                warm[:], warm[:], mybir.ActivationFunctionType.Exp, scale=1.0
            )

            # per-tile row-sums of exp; column b * nt + t
            parts = stat_pool.tile([128, 2 * nt], f32)

            for t in range(nt):
                c0 = t * tile_cols
                ftk = ft_pool.tile([128, 2, tile_cols], bf16)
                if t == 0:
                    # split the first load across queues so the first matmul
                    # group starts as soon as its 512-col slice lands
                    for s4 in range(0, tile_cols, 512):
                        nc.sync.dma_start(
                            ftk[:, :, s4 : s4 + 512],
                            ft[:, :, c0 + s4 : c0 + s4 + 512],
                        )
                else:
                    nc.sync.dma_start(ftk[:], ft[:, :, c0 : c0 + tile_cols])
                for b in range(2):
                    ps = psum_pool.tile([128, tile_cols], f32)
                    # kc outer so runs of 4 matmuls share one stationary
                    for kc in range(2):
                        for s in range(tile_cols // 512):
                            nc.tensor.matmul(
                                ps[:, s * 512 : (s + 1) * 512],
                                lhsT=xt_sb[:, kc, b * 128 : (b + 1) * 128],
                                rhs=ftk[:, kc, s * 512 : (s + 1) * 512],
                                start=(kc == 0),
                                stop=(kc == 1),
                            )
                    col = b * nt + t
                    # in-place exp keeps the (dead) exp values in PSUM; only
                    # accum_out (the row-sum) is consumed downstream
                    nc.scalar.activation(
                        ps[:],
                        ps[:],
                        mybir.ActivationFunctionType.Exp,
                        scale=1.0 / TEMP,
                        accum_out=parts[:, col : col + 1],
                    )

            nc.sync.dma_start(out[0], parts[:, 0:nt])
            nc.sync.dma_start(out[1], parts[:, nt : 2 * nt])
    nc.compile()
    return nc


def _get_nc():
    global _NC_CACHE
    if _NC_CACHE is None:
        _NC_CACHE = _build_nc()
    return _NC_CACHE


def _prep_ft(shard):
    """[NSHARD, D] f32 -> [128, 2, NPAD] bf16 with ft[p, kc, n] = shard[n, kc*128+p]."""
    out = np.zeros((128, 2, NPAD), BF16)
    blk_n = 4096
    for n0 in range(0, NSHARD, blk_n):
        blk = shard[n0 : n0 + blk_n].astype(BF16)  # [bn, 256] contiguous cast
        bt = blk.T  # [256, bn] view
        out[:, 0, n0 : n0 + blk.shape[0]] = bt[:128]
        out[:, 1, n0 : n0 + blk.shape[0]] = bt[128:]
    return out


def kernel(inputs, targets, features, _trace=False):
    global LAST_EXEC_NS
    from concourse.bass_utils import run_bass_kernel_spmd

    inputs = np.ascontiguousarray(np.asarray(inputs, dtype=np.float32))
    features = np.ascontiguousarray(np.asarray(features, dtype=np.float32))
    targets_np = np.asarray(targets)

    x = inputs / np.linalg.norm(inputs, axis=1, keepdims=True)
    xt16 = np.ascontiguousarray(x.T).astype(BF16).reshape(2, 128, B)

    try:
        from concurrent.futures import ThreadPoolExecutor

        with ThreadPoolExecutor(NCORES) as pool:
            fts = list(
                pool.map(
                    _prep_ft,
                    [features[k * NSHARD : (k + 1) * NSHARD] for k in range(NCORES)],
                )
            )
    except Exception:
        fts = [_prep_ft(features[k * NSHARD : (k + 1) * NSHARD]) for k in range(NCORES)]

    in_maps = [{"xt": xt16, "ft": fts[k]} for k in range(NCORES)]

    # overlap the (slow) full-bank host copy with the device round-trip
    from concurrent.futures import ThreadPoolExecutor as _TPE

    with _TPE(1) as _bg:
        copy_fut = _bg.submit(features.copy)
        res = run_bass_kernel_spmd(
            _get_nc(), in_maps, list(range(NCORES)), trace=_trace
        )
        new_features = copy_fut.result()
    LAST_EXEC_NS = res.exec_time_ns
    globals()["LAST_RESULTS"] = res

    # combine partial sumexps; padded columns contributed exp(0) = 1 each
    sumexp = np.zeros(B, dtype=np.float64)
    for r in res.results:
        sumexp += r["partials"].astype(np.float64).sum(axis=2).reshape(B)
    sumexp -= NCORES * (NPAD - NSHARD)
    lse = np.log(sumexp)

    tgt_logit = (
        np.sum(x.astype(np.float64) * features[targets_np].astype(np.float64), axis=1)
        / TEMP
    )
    loss = np.float32(np.mean(lse - tgt_logit))

    # sequential momentum scatter-update (duplicates applied in batch order)
    m = np.float32(MOMENTUM)
    om = np.float32(1.0 - MOMENTUM)
    for i in range(B):
        yi = int(targets_np[i])
        f = m * new_features[yi] + om * x[i]
        f = f / np.float32(np.sqrt(np.dot(f, f)))
        new_features[yi] = f
    return loss, new_features


# revision 16
# speedup vs baseline: 1.0028x; 1.0028x over previous
"""ClusterMemory (mode='CM') forward + memory-bank momentum update, Trainium2.

Strategy (8 NeuronCores, SPMD):
  - features [400000, 256] sharded row-wise: 50000 rows/core, shipped to the
    device pre-transposed ([256, n]) and cast to bf16 on the host so the
    device streams contiguous tiles straight into the matmul.
  - Per core: logits_tile = x_norm @ ft_shard (bf16 matmul, fp32 PSUM),
    fused exp(logit/TEMP) + row-sum on ScalarE (accum_out). Inputs are
    L2-normalized so |logit| <= 1/TEMP = 20 and sumexp fits fp32 with no
    max-subtraction pass.
  - Device returns only [2, 128, NT] partial row-sums per core (25.6 KB);
    host combines logsumexp in fp64, computes the target logits and the
    sequential 256-row momentum scatter (tiny) exactly in fp32, and
    assembles the full new_features.
"""

import numpy as np
import ml_dtypes

TEMP = 0.05
MOMENTUM = 0.2
B, D, N = 256, 256, 400000
NCORES = 8
NSHARD = N // NCORES        # 50000 feature rows per core
TILE_COLS = 2048            # columns per PSUM tile / ACT op
NT = 25                     # tiles per core
NPAD = TILE_COLS * NT       # 51200 (zero-padded; pad cols contribute exp(0)=1)
BF16 = ml_dtypes.bfloat16

_NC_CACHE = None
LAST_EXEC_NS = None
LAST_RESULTS = None


def _build_nc(tile_cols=TILE_COLS, nt=NT, bass_kwargs=None):
    import concourse.bass as bass
    import concourse.mybir as mybir
    import concourse.tile as tile
    from concourse import bacc
    from concourse._compat import get_trn_type

    npad = tile_cols * nt
    # Bacc (not plain Bass): its compile() runs generate_event_semaphores,
    # which splits multi-wait DMAs to satisfy TRN2's 1-wait-per-instruction
    # limit — walrus rejects the raw form with "Too many sync wait commands".
    nc = bacc.Bacc(
        get_trn_type() or "TRN2",
        target_bir_lowering=False,
        debug=False,
        num_devices=NCORES,
        **(bass_kwargs or {}),
    )
    bf16 = mybir.dt.bfloat16
    f32 = mybir.dt.float32

    xt = nc.declare_dram_parameter("xt", [2, 128, B], bf16, isOutput=False)
    # host layout: ft[p, kc, n] = features_shard.T[kc*128 + p, n]
    ft = nc.declare_dram_parameter("ft", [128, 2, npad], bf16, isOutput=False)
    out = nc.declare_dram_parameter("partials", [2, 128, nt], f32, isOutput=True)

    with tile.TileContext(nc) as tc:
        with (
            tc.tile_pool(name="const", bufs=1) as const_pool,
            tc.tile_pool(name="ftp", bufs=4) as ft_pool,
            tc.tile_pool(name="stat", bufs=1) as stat_pool,
            tc.tile_pool(name="psum", bufs=2, space=bass.MemorySpace.PSUM) as psum_pool,
        ):
            # x.T resident in SBUF: [d-chunk partitions, kc, b]
            xt_sb = const_pool.tile([128, 2, B], bf16)
            nc.sync.dma_start(xt_sb[:, 0, :], xt[0])
            nc.sync.dma_start(xt_sb[:, 1, :], xt[1])

            # warm the ACT exp table before the pipeline needs it
            warm = const_pool.tile([128, 1], f32)
            nc.gpsimd.memset(warm[:], 0.0)
            nc.scalar.activation(
                warm[:], warm[:], mybir.ActivationFunctionType.Exp, scale=1.0
            )

            # per-tile row-sums of exp; column b * nt + t
            parts = stat_pool.tile([128, 2 * nt], f32)

            for t in range(nt):
                c0 = t * tile_cols
                ftk = ft_pool.tile([128, 2, tile_cols], bf16)
                if t == 0:
                    # split the first load across queues so the first matmul
                    # group starts as soon as its 512-col slice lands
                    for s4 in range(0, tile_cols, 512):
                        nc.sync.dma_start(
                            ftk[:, :, s4 : s4 + 512],
                            ft[:, :, c0 + s4 : c0 + s4 + 512],
                        )
                else:
                    nc.sync.dma_start(ftk[:], ft[:, :, c0 : c0 + tile_cols])
                for b in range(2):
                    ps = psum_pool.tile([128, tile_cols], f32)
                    # kc outer so runs of 4 matmuls share one stationary
                    for kc in range(2):
                        for s in range(tile_cols // 512):
                            nc.tensor.matmul(
                                ps[:, s * 512 : (s + 1) * 512],
                                lhsT=xt_sb[:, kc, b * 128 : (b + 1) * 128],
                                rhs=ftk[:, kc, s * 512 : (s + 1) * 512],
                                start=(kc == 0),
                                stop=(kc == 1),
                            )
                    col = b * nt + t
                    # in-place exp keeps the (dead) exp values in PSUM; only
                    # accum_out (the row-sum) is consumed downstream
                    nc.scalar.activation(
                        ps[:],
                        ps[:],
                        mybir.ActivationFunctionType.Exp,
                        scale=1.0 / TEMP,
                        accum_out=parts[:, col : col + 1],
                    )

            nc.sync.dma_start(out[0], parts[:, 0:nt])
            nc.sync.dma_start(out[1], parts[:, nt : 2 * nt])
    nc.compile()
    return nc


def _get_nc():
    global _NC_CACHE
    if _NC_CACHE is None:
        _NC_CACHE = _build_nc()
    return _NC_CACHE


def _prep_ft(shard):
    """[NSHARD, D] f32 -> [128, 2, NPAD] bf16 with ft[p, kc, n] = shard[n, kc*128+p]."""
    out = np.zeros((128, 2, NPAD), BF16)
    blk_n = 4096
    for n0 in range(0, NSHARD, blk_n):
        blk = shard[n0 : n0 + blk_n].astype(BF16)  # [bn, 256] contiguous cast
        bt = blk.T  # [256, bn] view
        out[:, 0, n0 : n0 + blk.shape[0]] = bt[:128]
        out[:, 1, n0 : n0 + blk.shape[0]] = bt[128:]
    return out


def kernel(inputs, targets, features, _trace=False):
    global LAST_EXEC_NS
    from concourse.bass_utils import run_bass_kernel_spmd

    inputs = np.ascontiguousarray(np.asarray(inputs, dtype=np.float32))
    features = np.ascontiguousarray(np.asarray(features, dtype=np.float32))
    targets_np = np.asarray(targets)

    x = inputs / np.linalg.norm(inputs, axis=1, keepdims=True)
    xt16 = np.ascontiguousarray(x.T).astype(BF16).reshape(2, 128, B)

    try:
        from concurrent.futures import ThreadPoolExecutor

        with ThreadPoolExecutor(NCORES) as pool:
            fts = list(
                pool.map(
                    _prep_ft,
                    [features[k * NSHARD : (k + 1) * NSHARD] for k in range(NCORES)],
                )
            )
    except Exception:
        fts = [_prep_ft(features[k * NSHARD : (k + 1) * NSHARD]) for k in range(NCORES)]

    in_maps = [{"xt": xt16, "ft": fts[k]} for k in range(NCORES)]

    # overlap the (slow) full-bank host copy with the device round-trip
    from concurrent.futures import ThreadPoolExecutor as _TPE

    with _TPE(1) as _bg:
        copy_fut = _bg.submit(features.copy)
        res = run_bass_kernel_spmd(
            _get_nc(), in_maps, list(range(NCORES)), trace=_trace
        )
        new_features = copy_fut.result()
    LAST_EXEC_NS = res.exec_time_ns
    globals()["LAST_RESULTS"] = res

    # combine partial sumexps; padded columns contributed exp(0) = 1 each
    sumexp = np.zeros(B, dtype=np.float64)
    for r in res.results:
        sumexp += r["partials"].astype(np.float64).sum(axis=2).reshape(B)
    sumexp -= NCORES * (NPAD - NSHARD)
    lse = np.log(sumexp)

    tgt_logit = (
        np.sum(x.astype(np.float64) * features[targets_np].astype(np.float64), axis=1)
        / TEMP
    )
    loss = np.float32(np.mean(lse - tgt_logit))

    # sequential momentum scatter-update (duplicates applied in batch order)
    m = np.float32(MOMENTUM)
    om = np.float32(1.0 - MOMENTUM)
    for i in range(B):
        yi = int(targets_np[i])
        f = m * new_features[yi] + om * x[i]
        f = f / np.float32(np.sqrt(np.dot(f, f)))
        new_features[yi] = f
    return loss, new_features


# revision 21
# speedup vs baseline: 1.0208x; 1.0179x over previous
"""ClusterMemory (mode='CM') forward + memory-bank momentum update, Trainium2.

Strategy (8 NeuronCores, SPMD):
  - features [400000, 256] sharded row-wise: 50000 rows/core, shipped to the
    device pre-transposed ([256, n]) and cast to bf16 on the host so the
    device streams contiguous tiles straight into the matmul.
  - Per core: logits_tile = x_norm @ ft_shard (bf16 matmul, fp32 PSUM),
    fused exp(logit/TEMP) + row-sum on ScalarE (accum_out). Inputs are
    L2-normalized so |logit| <= 1/TEMP = 20 and sumexp fits fp32 with no
    max-subtraction pass.
  - Device returns only [2, 128, NT] partial row-sums per core (25.6 KB);
    host combines logsumexp in fp64, computes the target logits and the
    sequential 256-row momentum scatter (tiny) exactly in fp32, and
    assembles the full new_features.
"""

import numpy as np
import ml_dtypes

TEMP = 0.05
MOMENTUM = 0.2
B, D, N = 256, 256, 400000
NCORES = 8
NSHARD = N // NCORES        # 50000 feature rows per core
TILE_COLS = 2048            # columns per PSUM tile / ACT op
# 24 full tiles + ragged 848-col tail — no padding, no wasted exp work
COL_PLAN = [TILE_COLS] * (NSHARD // TILE_COLS) + (
    [NSHARD % TILE_COLS] if NSHARD % TILE_COLS else []
)
BF16 = ml_dtypes.bfloat16

_NC_CACHE = None
LAST_EXEC_NS = None
LAST_RESULTS = None


def _build_nc(col_plan=None, bass_kwargs=None):
    import concourse.bass as bass
    import concourse.mybir as mybir
    import concourse.tile as tile
    from concourse import bacc
    from concourse._compat import get_trn_type

    col_plan = COL_PLAN if col_plan is None else col_plan
    nt = len(col_plan)
    npad = sum(col_plan)
    # Bacc (not plain Bass): its compile() runs generate_event_semaphores,
    # which splits multi-wait DMAs to satisfy TRN2's 1-wait-per-instruction
    # limit — walrus rejects the raw form with "Too many sync wait commands".
    nc = bacc.Bacc(
        get_trn_type() or "TRN2",
        target_bir_lowering=False,
        debug=False,
        num_devices=NCORES,
        **(bass_kwargs or {}),
    )
    bf16 = mybir.dt.bfloat16
    f32 = mybir.dt.float32

    xt = nc.declare_dram_parameter("xt", [2, 128, B], bf16, isOutput=False)
    # host layout: ft[p, kc, n] = features_shard.T[kc*128 + p, n]
    ft = nc.declare_dram_parameter("ft", [128, 2, npad], bf16, isOutput=False)
    out = nc.declare_dram_parameter("partials", [2, 128, nt], f32, isOutput=True)

    with tile.TileContext(nc) as tc:
        with (
            tc.tile_pool(name="const", bufs=1) as const_pool,
            tc.tile_pool(name="ftp", bufs=4) as ft_pool,
            tc.tile_pool(name="stat", bufs=1) as stat_pool,
            tc.tile_pool(name="psum", bufs=2, space=bass.MemorySpace.PSUM) as psum_pool,
        ):
            # x.T resident in SBUF: [d-chunk partitions, kc, b]
            xt_sb = const_pool.tile([128, 2, B], bf16)
            nc.sync.dma_start(xt_sb[:, 0, :], xt[0])
            nc.sync.dma_start(xt_sb[:, 1, :], xt[1])

            # warm the ACT exp table before the pipeline needs it
            warm = const_pool.tile([128, 1], f32)
            nc.gpsimd.memset(warm[:], 0.0)
            nc.scalar.activation(
                warm[:], warm[:], mybir.ActivationFunctionType.Exp, scale=1.0
            )

            # per-tile row-sums of exp; column b * nt + t
            parts = stat_pool.tile([128, 2 * nt], f32)

            c0 = 0
            for t, cols in enumerate(col_plan):
                ftk = ft_pool.tile([128, 2, cols], bf16, tag="ftk")
                if t == 0:
                    # split the first load across queues so the first matmul
                    # group starts as soon as its 512-col slice lands
                    for s4 in range(0, cols, 512):
                        w4 = min(512, cols - s4)
                        nc.sync.dma_start(
                            ftk[:, :, s4 : s4 + w4],
                            ft[:, :, c0 + s4 : c0 + s4 + w4],
                        )
                else:
                    nc.sync.dma_start(ftk[:], ft[:, :, c0 : c0 + cols])
                for b in range(2):
                    ps = psum_pool.tile([128, cols], f32, tag="ps")
                    # kc outer so runs of matmuls share one stationary
                    for kc in range(2):
                        for s in range(0, cols, 512):
                            w = min(512, cols - s)
                            nc.tensor.matmul(
                                ps[:, s : s + w],
                                lhsT=xt_sb[:, kc, b * 128 : (b + 1) * 128],
                                rhs=ftk[:, kc, s : s + w],
                                start=(kc == 0),
                                stop=(kc == 1),
                            )
                    col = b * nt + t
                    # in-place exp keeps the (dead) exp values in PSUM; only
                    # accum_out (the row-sum) is consumed downstream
                    nc.scalar.activation(
                        ps[:],
                        ps[:],
                        mybir.ActivationFunctionType.Exp,
                        scale=1.0 / TEMP,
                        accum_out=parts[:, col : col + 1],
                    )
                c0 += cols

            nc.sync.dma_start(out[0], parts[:, 0:nt])
            nc.sync.dma_start(out[1], parts[:, nt : 2 * nt])
    nc.compile()
    return nc


def _get_nc():
    global _NC_CACHE
    if _NC_CACHE is None:
        _NC_CACHE = _build_nc()
    return _NC_CACHE


def _prep_ft(shard):
    """[NSHARD, D] f32 -> [128, 2, NSHARD] bf16 with ft[p, kc, n] = shard[n, kc*128+p]."""
    out = np.empty((128, 2, NSHARD), BF16)
    blk_n = 4096
    for n0 in range(0, NSHARD, blk_n):
        blk = shard[n0 : n0 + blk_n].astype(BF16)  # [bn, 256] contiguous cast
        bt = blk.T  # [256, bn] view
        out[:, 0, n0 : n0 + blk.shape[0]] = bt[:128]
        out[:, 1, n0 : n0 + blk.shape[0]] = bt[128:]
    return out


def kernel(inputs, targets, features, _trace=False):
    global LAST_EXEC_NS
    from concourse.bass_utils import run_bass_kernel_spmd

    inputs = np.ascontiguousarray(np.asarray(inputs, dtype=np.float32))
    features = np.ascontiguousarray(np.asarray(features, dtype=np.float32))
    targets_np = np.asarray(targets)

    x = inputs / np.linalg.norm(inputs, axis=1, keepdims=True)
    xt16 = np.ascontiguousarray(x.T).astype(BF16).reshape(2, 128, B)

    try:
        from concurrent.futures import ThreadPoolExecutor

        with ThreadPoolExecutor(NCORES) as pool:
            fts = list(
                pool.map(
                    _prep_ft,
                    [features[k * NSHARD : (k + 1) * NSHARD] for k in range(NCORES)],
                )
            )
    except Exception:
        fts = [_prep_ft(features[k * NSHARD : (k + 1) * NSHARD]) for k in range(NCORES)]

    in_maps = [{"xt": xt16, "ft": fts[k]} for k in range(NCORES)]

    # overlap the (slow) full-bank host copy with the device round-trip
    from concurrent.futures import ThreadPoolExecutor as _TPE

    with _TPE(1) as _bg:
        copy_fut = _bg.submit(features.copy)
        res = run_bass_kernel_spmd(
            _get_nc(), in_maps, list(range(NCORES)), trace=_trace
        )
        new_features = copy_fut.result()
    LAST_EXEC_NS = res.exec_time_ns
    globals()["LAST_RESULTS"] = res

    # combine partial sumexps across shards
    sumexp = np.zeros(B, dtype=np.float64)
    for r in res.results:
        sumexp += r["partials"].astype(np.float64).sum(axis=2).reshape(B)
    lse = np.log(sumexp)

    tgt_logit = (
        np.sum(x.astype(np.float64) * features[targets_np].astype(np.float64), axis=1)
        / TEMP
    )
    loss = np.float32(np.mean(lse - tgt_logit))

    # sequential momentum scatter-update (duplicates applied in batch order)
    m = np.float32(MOMENTUM)
    om = np.float32(1.0 - MOMENTUM)
    for i in range(B):
        yi = int(targets_np[i])
        f = m * new_features[yi] + om * x[i]
        f = f / np.float32(np.sqrt(np.dot(f, f)))
        new_features[yi] = f
    return loss, new_features
